# revision 16
# baseline (speedup 1.0000x reference)
"""Causal multi-head attention (B=2, T=2048, E=1024, 16 heads) on 8 TRN2 cores.

Sharding: 8-way tensor-parallel over heads (2 heads/core) for QKV projections
and attention; one AllToAll per head re-shards the attention output over
tokens so each core computes the output projection for its 512-token block.

All matmuls run in float32r (TF32-like, full PE rate at free-dim >= 256).
The host passes x^T and the weight transposes directly as float32r inputs,
so no on-device transposes are needed except for V (computed on device).
Scores are computed transposed (S^T = K Q^T, [k-toks x q-toks]) so softmax
P^T feeds the AV matmul directly; a ones column appended to V makes the AV
matmul emit softmax denominators; causal masking is one 128x128 triangle
add per diagonal block plus block-level skipping; max-subtraction is
omitted (scores are O(1), exp cannot overflow).
"""
import sys

if "/opt/trn_rl_repo" not in sys.path:
    sys.path.insert(0, "/opt/trn_rl_repo")

import numpy as np

import concourse.bacc as bacc
import concourse.mybir as mybir
from concourse import tile
from concourse.bass_utils import run_bass_kernel_spmd

dt = mybir.dt
AF = mybir.ActivationFunctionType
ALU = mybir.AluOpType

B, T, E, HS, NH = 2, 2048, 1024, 64, 16
NCORE = 8
NTOK = B * T            # 4096
CH = 512                # token chunk
NCH = NTOK // CH        # 8
CPB = NCH // B          # chunks per batch = 4
SUB = 128
NSUB = CH // SUB        # 4
NEG = -1.0e30

_nc_cache = {}


def build_nc(stage="full"):
    nc = bacc.Bacc("TRN2", target_bir_lowering=False, debug=False,
                   num_devices=NCORE)
    f32, f32r = dt.float32, dt.float32r

    xT = nc.declare_dram_parameter("xT", [E, NTOK], f32r, isOutput=False)
    wqT = nc.declare_dram_parameter("wqT", [E, 128], f32r, isOutput=False)
    wkT = nc.declare_dram_parameter("wkT", [E, 128], f32r, isOutput=False)
    wvT = nc.declare_dram_parameter("wvT", [E, 128], f32r, isOutput=False)
    woT = nc.declare_dram_parameter("woT", [E, E], f32r, isOutput=False)
    bqs = nc.declare_dram_parameter("bqs", [128, 1], f32, isOutput=False)
    bks = nc.declare_dram_parameter("bks", [128, 1], f32, isOutput=False)
    bvs = nc.declare_dram_parameter("bvs", [128, 1], f32, isOutput=False)
    bo_b = nc.declare_dram_parameter("bo_b", [128, E], f32, isOutput=False)
    eye = nc.declare_dram_parameter("eye", [128, 128], f32, isOutput=False)
    tri = nc.declare_dram_parameter("tri", [128, 128], f32, isOutput=False)
    ones_v = nc.declare_dram_parameter("ones_v", [128, NCH * NSUB], f32,
                                       isOutput=False)
    ones_r = nc.declare_dram_parameter("ones_r", [1, 64], f32, isOutput=False)
    y = nc.declare_dram_parameter("y", [CH, E], f32, isOutput=True)

    with tile.TileContext(nc) as tc:
        from contextlib import ExitStack
        with ExitStack() as top:
            const = top.enter_context(tc.tile_pool(name="const", bufs=1))
            persist = top.enter_context(tc.tile_pool(name="persist", bufs=1))
            xtp_pool = top.enter_context(tc.tile_pool(name="xtp", bufs=3))
            ps_t = top.enter_context(
                tc.tile_pool(name="ps_t", bufs=1, space="PSUM"))
            ps_q = top.enter_context(
                tc.tile_pool(name="ps_q", bufs=2, space="PSUM"))
            ps_s = top.enter_context(
                tc.tile_pool(name="ps_s", bufs=3, space="PSUM"))
            ps_a = top.enter_context(
                tc.tile_pool(name="ps_a", bufs=2, space="PSUM"))
            dram = top.enter_context(
                tc.tile_pool(name="dram", bufs=1, space="DRAM"))

            # ---- constants -------------------------------------------------
            eye_sb = const.tile([128, 128], f32, name="eye_sb")
            nc.sync.dma_start(eye_sb[:], eye[:])
            eyer_sb = const.tile([128, 128], f32r, name="eyer_sb")
            nc.vector.tensor_copy(eyer_sb[:], eye_sb[:])
            tri_sb = const.tile([128, 128], f32, name="tri_sb")
            nc.sync.dma_start(tri_sb[:], tri[:])
            onesv_sb = const.tile([128, NCH * NSUB], f32, name="onesv_sb")
            nc.sync.dma_start(onesv_sb[:], ones_v[:])
            onesr_sb = const.tile([1, 64], f32, name="onesr_sb")
            nc.sync.dma_start(onesr_sb[:], ones_r[:])
            onesr_r = const.tile([1, 64], f32r, name="onesr_r")
            nc.vector.tensor_copy(onesr_r[:], onesr_sb[:])
            bq_sb = const.tile([128, 1], f32, name="bq_sb")
            nc.sync.dma_start(bq_sb[:], bqs[:])
            bk_sb = const.tile([128, 1], f32, name="bk_sb")
            nc.sync.dma_start(bk_sb[:], bks[:])
            bv_sb = const.tile([128, 1], f32, name="bv_sb")
            nc.sync.dma_start(bv_sb[:], bvs[:])
            bo_sb = const.tile([128, E], f32, name="bo_sb")
            nc.sync.dma_start(bo_sb[:], bo_b[:])

            # ---- persistent tensors ---------------------------------------
            wq_sb = persist.tile([128, 8, 128], f32r, name="wq_sb")
            wk_sb = persist.tile([128, 8, 128], f32r, name="wk_sb")
            wv_sb = persist.tile([128, 8, 128], f32r, name="wv_sb")
            wo_sb = persist.tile([128, 8, E], f32r, name="wo_sb")
            nc.sync.dma_start(wq_sb[:], wqT.rearrange("(e p) m -> p e m", p=128))
            nc.sync.dma_start(wk_sb[:], wkT.rearrange("(e p) m -> p e m", p=128))
            nc.sync.dma_start(wv_sb[:], wvT.rearrange("(e p) m -> p e m", p=128))
            kT = persist.tile([128, NCH, CH], f32r, name="kT")
            qT = persist.tile([128, NCH, CH], f32r, name="qT")
            vh0 = persist.tile([128, NCH * NSUB, 65], f32r, name="vh0")
            vh1 = persist.tile([128, NCH * NSUB, 65], f32r, name="vh1")

            cc_in = [dram.tile([NCH, 64, CH], f32r, name=f"cc_in{h}")
                     for h in range(2)]
            cc_out = [dram.tile([NCH, 64, CH], f32r, name=f"cc_out{h}")
                      for h in range(2)]

            # ones column of the augmented V
            nc.vector.tensor_copy(vh0[:, :, 64], onesv_sb[:])
            nc.vector.tensor_copy(vh1[:, :, 64], onesv_sb[:])

            # ---- phase B: Q^T, K^T, V per chunk ---------------------------
            with tc.tile_pool(name="vstage", bufs=2) as vstage:
                for t in range(NCH):
                    xTt = xtp_pool.tile([128, 8, CH], f32r, name="xTt",
                                        tag="xTt")
                    for e in range(8):
                        for half in range(2):
                            nc.sync.dma_start(
                                xTt[:, e, 256 * half:256 * (half + 1)],
                                xT[128 * e:128 * (e + 1),
                                   CH * t + 256 * half:CH * t + 256 * (half + 1)])

                    # Q^T (scale 1/8 folded), K^T
                    for wsb, bias, scale, dest in (
                            (wq_sb, bq_sb, 0.125, qT),
                            (wk_sb, bk_sb, 1.0, kT)):
                        ps = ps_q.tile([128, CH], f32, name="psqk", tag="psq")
                        for e in range(8):
                            nc.tensor.matmul(ps[:], wsb[:, e, :], xTt[:, e, :],
                                             start=(e == 0), stop=(e == 7))
                        if scale == 1.0:
                            nc.vector.tensor_scalar_add(dest[:, t, :], ps[:],
                                                        bias[:])
                        else:
                            nc.vector.tensor_scalar(
                                dest[:, t, :], ps[:], scale, bias[:],
                                ALU.mult, ALU.add)

                    # V^T then transpose to V rows, split per head
                    psv = ps_q.tile([128, CH], f32, name="psv", tag="psq")
                    for e in range(8):
                        nc.tensor.matmul(psv[:], wv_sb[:, e, :], xTt[:, e, :],
                                         start=(e == 0), stop=(e == 7))
                    vTs = vstage.tile([128, CH], f32r, name="vTs", tag="vTs")
                    nc.vector.tensor_scalar_add(vTs[:], psv[:], bv_sb[:])
                    for s in range(NSUB):
                        tv = ps_q.tile([128, 512], f32r, name="tpv",
                                       tag="psq")
                        nc.tensor.transpose(
                            tv[:, 0:128], vTs[:, 128 * s:128 * (s + 1)],
                            eyer_sb[:])
                        g = NSUB * t + s
                        nc.vector.tensor_copy(vh0[:, g, 0:64], tv[:, 0:64])
                        nc.vector.tensor_copy(vh1[:, g, 0:64], tv[:, 64:128])

            for r in range(8):
                nc.sync.dma_start(
                    wo_sb[:, r, :],
                    woT[128 * r:128 * (r + 1), :])

            if stage == "qkv":
                yv = y.rearrange("(s p) e -> p s e", p=128)
                dbg = persist.tile([128, 8, CH], f32, name="dbg")
                nc.vector.tensor_copy(dbg[:], qT[:].bitcast(f32))
                nc.sync.dma_start(yv, dbg.rearrange("p c t -> p (c t)").rearrange("p (s e) -> p s e", s=4))

            # ---- phase C: attention, head-major ----------------------------
            with tc.tile_pool(name="ppool", bufs=4) as ppool, \
                 tc.tile_pool(name="apool", bufs=2) as apool:
                for h in (range(2) if stage != "qkv" else []):
                    vh = vh0 if h == 0 else vh1
                    pb = 64 * h
                    for t in range(NCH):
                        b0 = CPB * (t // CPB)
                        a_ps = ps_a.tile([128, CH], f32, name="a_ps", tag="aps")
                        def emit_scores(kc):
                            diag = kc == t
                            pT = ppool.tile([128, NSUB, CH], f32r,
                                            name="pT", tag="pT")
                            for s in range(NSUB):
                                q0 = 128 * s if diag else 0
                                sps = ps_s.tile([128, CH], f32,
                                                name="sps", tag="sps")
                                nc.tensor.matmul(
                                    sps[:, q0:CH],
                                    kT[pb:pb + 64, kc, 128 * s:128 * (s + 1)],
                                    qT[pb:pb + 64, t, q0:CH],
                                    start=True, stop=True)
                                if diag:
                                    nc.vector.tensor_add(
                                        sps[:, q0:q0 + 128],
                                        sps[:, q0:q0 + 128], tri_sb[:])
                                nc.scalar.activation(
                                    pT[:, s, q0:CH], sps[:, q0:CH], AF.Exp)
                            return pT

                        def emit_av(kc, pT):
                            diag = kc == t
                            for s in range(NSUB):
                                q0 = 128 * s if diag else 0
                                g = NSUB * kc + s
                                nc.tensor.matmul(
                                    a_ps[0:65, q0:CH], vh[:, g, :],
                                    pT[:, s, q0:CH],
                                    start=(kc == b0 and s == 0),
                                    stop=(diag and s == NSUB - 1))

                        prev = None
                        for kc in range(b0, t + 1):
                            pT = emit_scores(kc)
                            if prev is not None:
                                emit_av(*prev)
                            prev = (kc, pT)
                        emit_av(*prev)
                        # normalize: recip of sums row, broadcast via PE
                        rec = apool.tile([1, CH], f32r, name="rec", tag="rec")
                        with nc.allow_low_precision(
                                reason="f32r recip feeds PE broadcast; "
                                       "psum accum stays fp32"):
                            nc.vector.reciprocal(rec[:], a_ps[64:65, :])
                        bc_ps = ps_t.tile([64, CH], f32, name="bc_ps",
                                          tag="pst", bufs=1)
                        nc.tensor.matmul(bc_ps[:], onesr_r[:], rec[:],
                                         start=True, stop=True)
                        bc_sb = apool.tile([64, CH], f32r, name="bc_sb",
                                           tag="bcs")
                        nc.vector.tensor_copy(bc_sb[:], bc_ps[:])
                        a_sb = apool.tile([64, CH], f32r, name="a_sb",
                                          tag="asb")
                        nc.vector.tensor_mul(a_sb[:], a_ps[0:64, :], bc_sb[:])
                        nc.sync.dma_start(cc_in[h][t, :, :], a_sb[:])
                    nc.gpsimd.collective_compute(
                        "AllToAll", ALU.bypass,
                        ins=[cc_in[h].opt()], outs=[cc_out[h].opt()],
                        replica_groups=[list(range(NCORE))])

            if stage == "attn":
                yv = y.rearrange("(s p) e -> p s e", p=128)
                for h in range(2):
                    for c in range(NCH):
                        nc.sync.dma_start(
                            yv[64 * h:64 * (h + 1), c // 2,
                               (c % 2) * 512:(c % 2) * 512 + 512],
                            cc_in[h][c, :, :].bitcast(f32))

            # ---- phase E: output projection on this core's token block -----
            # split by head: the h0 half runs as soon as A2A#0 lands and
            # overlaps h1 attention + A2A#1; the h1 half adds bias and stores.
            with tc.tile_pool(name="ystage", bufs=2) as ystage, \
                 tc.tile_pool(name="yacc_pool", bufs=1) as yacc_pool:
                if stage == "full":
                    aT = xtp_pool.tile([128, 8, CH], f32r, name="aT",
                                       tag="xTt")
                    yacc = yacc_pool.tile([128, NSUB, E], f32, name="yacc")
                    for kt in range(8):
                        nc.sync.dma_start(aT[0:64, kt, :], cc_out[0][kt, :, :])
                    for m in range(NSUB):
                        for nch in range(2):
                            yps = ps_t.tile([128, 512], f32, name="yps",
                                            tag="pst", bufs=1)
                            for kt in range(8):
                                nc.tensor.matmul(
                                    yps[:],
                                    aT[0:64, kt, 128 * m:128 * (m + 1)],
                                    wo_sb[0:64, kt,
                                          512 * nch:512 * (nch + 1)],
                                    start=(kt == 0), stop=(kt == 7))
                            nc.vector.tensor_add(
                                yacc[:, m, 512 * nch:512 * (nch + 1)], yps[:],
                                bo_sb[:, 512 * nch:512 * (nch + 1)])
                    for kt in range(8):
                        nc.sync.dma_start(aT[64:128, kt, :],
                                          cc_out[1][kt, :, :])
                for m in (range(NSUB) if stage == "full" else []):
                    for nch in range(2):
                        yps = ps_t.tile([128, 512], f32, name="yps",
                                        tag="pst", bufs=1)
                        for kt in range(8):
                            nc.tensor.matmul(
                                yps[:], aT[64:128, kt, 128 * m:128 * (m + 1)],
                                wo_sb[64:128, kt, 512 * nch:512 * (nch + 1)],
                                start=(kt == 0), stop=(kt == 7))
                        ysb = ystage.tile([128, 512], f32, name="ysb",
                                          tag="ysb")
                        nc.vector.tensor_add(
                            ysb[:], yps[:],
                            yacc[:, m, 512 * nch:512 * (nch + 1)])
                        nc.sync.dma_start(
                            y[128 * m:128 * (m + 1),
                              512 * nch:512 * (nch + 1)],
                            ysb[:])
    nc.compile()
    return nc


def _prep_in_maps(embd_q, Wq, bq, Wk, bk, Wv, bv, Wo, bo):
    x = embd_q.reshape(NTOK, E).astype(np.float32)
    xT = np.ascontiguousarray(x.T)
    eye = np.eye(128, dtype=np.float32)
    r = np.arange(128)
    tri = np.where(r[:, None] > r[None, :], np.float32(NEG), np.float32(0.0))
    tri = np.ascontiguousarray(tri, dtype=np.float32)
    ones_v = np.ones((128, NCH * NSUB), dtype=np.float32)
    ones_r = np.ones((1, 64), dtype=np.float32)
    bo_b = np.ascontiguousarray(
        np.broadcast_to(bo.astype(np.float32), (128, E)))
    woT = np.ascontiguousarray(Wo.astype(np.float32).T)
    in_maps = []
    for c in range(NCORE):
        sl = slice(128 * c, 128 * (c + 1))
        in_maps.append({
            "xT": xT,
            "wqT": np.ascontiguousarray(Wq[sl].astype(np.float32).T),
            "wkT": np.ascontiguousarray(Wk[sl].astype(np.float32).T),
            "wvT": np.ascontiguousarray(Wv[sl].astype(np.float32).T),
            "woT": woT,
            "bqs": np.ascontiguousarray(
                (bq[sl] * 0.125).reshape(128, 1), dtype=np.float32),
            "bks": np.ascontiguousarray(bk[sl].reshape(128, 1),
                                        dtype=np.float32),
            "bvs": np.ascontiguousarray(bv[sl].reshape(128, 1),
                                        dtype=np.float32),
            "bo_b": bo_b,
            "eye": eye,
            "tri": tri,
            "ones_v": ones_v,
            "ones_r": ones_r,
        })
    return in_maps


def kernel(embd_q, Wq, bq, Wk, bk, Wv, bv, Wo, bo, _trace=False,
           _stage="full"):
    if _stage not in _nc_cache:
        _nc_cache[_stage] = build_nc(_stage)
    in_maps = _prep_in_maps(np.asarray(embd_q), np.asarray(Wq), np.asarray(bq),
                            np.asarray(Wk), np.asarray(bk), np.asarray(Wv),
                            np.asarray(bv), np.asarray(Wo), np.asarray(bo))
    import os
    tc_env = os.environ.get("TRACE_CORES")
    res = run_bass_kernel_spmd(
        _nc_cache[_stage], in_maps, list(range(NCORE)), trace=_trace,
        trace_cores=(list(range(NCORE)) if tc_env else None))
    out = np.concatenate(
        [res.results[c]["y"] for c in range(NCORE)], axis=0)
    out = out.reshape(B, T, E)
    kernel.last_results = res
    return out
